# revision 13
# baseline (speedup 1.0000x reference)
"""Local (banded) attention -> mean over sequence, on 8 TRN2 NeuronCores.

Math: out[b] = mean_i softmax_j(masked(q_i . k_j / sqrt(H)))-weighted v_j
Reductions used (exact up to softmax shift invariance):
  1. scores'[i,j] = qa_i . x_j with qa = x @ A + cb,
     A = Wq Wk^T / sqrt(H), cb = Wk bq / sqrt(H)
     (terms constant in j drop out of the softmax).
  2. mean_i ctx_i = (1/S) sum_j tw_j v_j with tw_j = sum_i w_ij, and since
     sum_j tw_j = S:  out = (u/S) @ Wv + bv with u = sum_j tw_j x_j.
So the device kernel only computes qa, banded exp-scores, per-key total
weights tw, and u = tw @ x_slice.  The [4,256]@[256,256] epilogue runs on host.

Sharding: 8 cores = batch(4) x sequence-half(2); each core owns 2048 query
rows and a symmetric 128-row halo key range (zero-padded outside the
sequence).  Zero-padded keys contribute exp(0)=1 to each edge query's row
sum; that count is exact and is subtracted via the reduce-init operand of
tensor_tensor_reduce.  Padded keys contribute 0 to u (x row is 0), so the
result is exact.  Per-core partial u vectors sum on the host (u is linear
in tw).
"""

import numpy as np
import ml_dtypes

B, S, H = 4, 4096, 256
W = 128          # window size this kernel is specialized for
SH = S // 2      # query rows per core
HALO = 128
NK = SH + 2 * HALO   # keys per core incl. zero-padded halo
NKC = NK // 128      # 18 key chunks
NQB = SH // 128      # 16 query blocks
BF16 = ml_dtypes.bfloat16

_CACHE = {}


def _build():
    import concourse.bass as bass
    import concourse.tile as tile
    import concourse.mybir as mybir
    from concourse import bacc

    f32 = mybir.dt.float32
    bf16 = mybir.dt.bfloat16

    nc = bacc.Bacc(
        "TRN2", target_bir_lowering=False, debug=False,
        enable_asserts=False, num_devices=1,
    )

    xT_d = nc.dram_tensor("xT", [H, NK], bf16, kind="ExternalInput").ap()
    xn_d = nc.dram_tensor("xn", [NK, H], bf16, kind="ExternalInput").ap()
    a_d = nc.dram_tensor("a", [H, H], bf16, kind="ExternalInput").ap()
    cb_d = nc.dram_tensor("cb", [128, 2], f32, kind="ExternalInput").ap()
    mk_d = nc.dram_tensor("mk", [128, 384], bf16, kind="ExternalInput").ap()
    rc_d = nc.dram_tensor("rc", [128, NQB], f32, kind="ExternalInput").ap()
    u_d = nc.dram_tensor("u", [1, 256], f32, kind="ExternalOutput").ap()

    with tile.TileContext(nc) as tc:
        with (
            tc.tile_pool(name="cst", bufs=1) as cst,
            tc.tile_pool(name="big", bufs=1) as big,
            tc.tile_pool(name="wrk", bufs=4) as wrk,
            tc.tile_pool(name="pqa", bufs=1, space="PSUM") as pqa,
            tc.tile_pool(name="psc", bufs=2, space="PSUM") as psc,
            tc.tile_pool(name="ptw", bufs=1, space="PSUM") as ptw,
            tc.tile_pool(name="pu", bufs=1, space="PSUM") as pu,
        ):
            a0 = cst.tile([128, 256], bf16, tag="a0")
            a1 = cst.tile([128, 256], bf16, tag="a1")
            cb_sb = cst.tile([128, 2], f32, tag="cb")
            mk_sb = cst.tile([128, 384], bf16, tag="mk")
            rc_sb = cst.tile([128, NQB], f32, tag="rc")
            xT0 = big.tile([128, NK], bf16, tag="xT0")
            xT1 = big.tile([128, NK], bf16, tag="xT1")
            xn_sb = big.tile([128, NKC * 256], bf16, tag="xn")
            qa0 = big.tile([128, SH], bf16, tag="qa0")
            qa1 = big.tile([128, SH], bf16, tag="qa1")
            twT_sb = cst.tile([128, NKC], bf16, tag="twT")
            u_sb = cst.tile([1, 256], f32, tag="u")

            nc.sync.dma_start(a0[:], a_d[0:128, :])
            nc.sync.dma_start(a1[:], a_d[128:256, :])
            nc.sync.dma_start(cb_sb[:], cb_d[:])
            nc.sync.dma_start(mk_sb[:], mk_d[:])
            nc.sync.dma_start(rc_sb[:], rc_d[:])
            ones_bf = cst.tile([128, 1], bf16, tag="ones")
            nc.gpsimd.memset(ones_bf[:], 1.0)
            nc.sync.dma_start(xT0[:], xT_d[0:128, :])
            nc.sync.dma_start(xT1[:], xT_d[128:256, :])
            xn_v = xn_sb.rearrange("p (c d) -> p c d", d=256)
            xnd_v = xn_d.rearrange("(c p) d -> p c d", p=128)
            for g in range(3):
                nc.sync.dma_start(xn_v[:, 6 * g:6 * (g + 1), :],
                                  xnd_v[:, 6 * g:6 * (g + 1), :])

            qa = (qa0, qa1)
            xT = (xT0, xT1)
            a = (a0, a1)

            # qa projection: qaT[m] = sum_k A[k, m-chunk].T @ xT[k][:, queries]
            # (m, k) fixed across the n sweep so each weight loads once
            for m in range(2):
                pss = [pqa.tile([128, 512], f32, tag=f"pqa{n}", name=f"pqa{m}_{n}")
                       for n in range(SH // 512)]
                for k in range(2):
                    for n in range(SH // 512):
                        nc.tensor.matmul(
                            pss[n][:],
                            a[k][:, m * 128:(m + 1) * 128],
                            xT[k][:, HALO + n * 512: HALO + (n + 1) * 512],
                            start=(k == 0), stop=(k == 1),
                        )
                for n in range(SH // 512):
                    nc.scalar.activation(
                        qa[m][:, n * 512:(n + 1) * 512], pss[n][:],
                        mybir.ActivationFunctionType.Identity,
                        bias=cb_sb[:, m:m + 1],
                    )

            twp = ptw.tile([128, NKC], f32, tag="tw")
            up = pu.tile([1, 256], f32, tag="u")
            # query block i attends local key band [128*i, 128*i+384)
            em_live = {}
            ivb_live = {}

            def emit_chunk(jc):
                # key chunk jc accumulates from blocks jc-2..jc; the group
                # opens and closes before the next chunk's group starts
                # (sim requires one pending PSUM group per zero region).
                blocks = [i for i in range(jc - 2, jc + 1) if 0 <= i < NQB]
                for i in blocks:
                    nc.tensor.matmul(
                        twp[:, jc:jc + 1],
                        em_live[i][:, (jc - i) * 128:(jc - i + 1) * 128],
                        ones_bf[:],
                        start=(i == blocks[0]), stop=(i == blocks[-1]),
                    )
                if jc % 3 == 2 or jc == NKC - 1:
                    g0 = (jc // 3) * 3
                    nc.scalar.copy(twT_sb[:, g0:jc + 1], twp[:, g0:jc + 1])
                    for j2 in range(g0, jc + 1):
                        nc.tensor.matmul(
                            up[:],
                            twT_sb[:, j2:j2 + 1],
                            xn_sb[:, j2 * 256:(j2 + 1) * 256],
                            start=(j2 == 0), stop=(j2 == NKC - 1),
                        )

            rs_all = cst.tile([128, NQB], f32, tag="rs_all")
            iv_all = cst.tile([128, NQB], f32, tag="iv_all")
            GB = 4  # reciprocal batch
            for g in range(NQB // GB):
                ems = {}
                for i in range(g * GB, (g + 1) * GB):
                    c0 = 128 * i
                    ps = psc.tile([128, 384], f32, tag="psc")
                    for k in range(2):
                        nc.tensor.matmul(
                            ps[:],
                            qa[k][:, i * 128:(i + 1) * 128],
                            xT[k][:, c0:c0 + 384],
                            start=(k == 0), stop=(k == 1),
                        )
                    ex = wrk.tile([128, 384], bf16, tag="ex")
                    nc.scalar.activation(
                        ex[:], ps[:], mybir.ActivationFunctionType.Exp,
                    )
                    em = wrk.tile([128, 384], bf16, tag=f"em{i % (GB + 1)}",
                                  name=f"em_{i}")
                    rs0 = wrk.tile([128, 1], f32, tag="rs0")
                    nc.vector.scalar_tensor_tensor(
                        em[:], ex[:], 1.0, mk_sb[:],
                        mybir.AluOpType.mult, mybir.AluOpType.mult,
                        accum_out=rs0[:],
                    )
                    nc.vector.tensor_scalar_add(
                        rs_all[:, i:i + 1], rs0[:], rc_sb[:, i:i + 1])
                    ems[i] = em
                gs = slice(g * GB, (g + 1) * GB)
                nc.vector.reciprocal(iv_all[:, gs], rs_all[:, gs])
                for i in range(g * GB, (g + 1) * GB):
                    emn = wrk.tile([128, 384], bf16, tag=f"emn{i % (GB + 1)}",
                                   name=f"emn_{i}")
                    nc.vector.tensor_scalar_mul(
                        emn[:], ems[i][:], iv_all[:, i:i + 1])
                    em_live[i] = emn
                for i in range(g * GB, (g + 1) * GB):
                    emit_chunk(i)
                    if i == NQB - 1:
                        emit_chunk(i + 1)
                        emit_chunk(i + 2)

            nc.scalar.copy(u_sb[:], up[:])
            nc.sync.dma_start(u_d[:], u_sb[:])

    nc.compile()
    return nc


def _numpy_fallback(x, Wq, bq, Wk, bk, Wv, bv, window_size):
    out = np.zeros((B, H), np.float64)
    xs = x.astype(np.float64)
    A = (Wq.astype(np.float64) @ Wk.astype(np.float64).T) / np.sqrt(H)
    cb = (Wk.astype(np.float64) @ bq.astype(np.float64)) / np.sqrt(H)
    idx = np.arange(x.shape[1])
    band = np.abs(idx[:, None] - idx[None, :]) <= int(window_size)
    for b in range(x.shape[0]):
        qa = xs[b] @ A + cb
        sc = qa @ xs[b].T
        e = np.exp(sc - sc.max(axis=-1, keepdims=True)) * band
        w = e / e.sum(-1, keepdims=True)
        tw = w.sum(axis=0)
        out[b] = (tw @ xs[b] / x.shape[1]) @ Wv.astype(np.float64) + bv
    return out.astype(np.float32)


def kernel(x, Wq, bq, Wk, bk, Wv, bv, window_size):
    x = np.asarray(x)
    Wq, bq = np.asarray(Wq), np.asarray(bq)
    Wk, bk = np.asarray(Wk), np.asarray(bk)
    Wv, bv = np.asarray(Wv), np.asarray(bv)
    if int(window_size) != W or x.shape != (B, S, H):
        return _numpy_fallback(x, Wq, bq, Wk, bk, Wv, bv, window_size)

    from concourse.bass_utils import run_bass_kernel_spmd

    if "nc" not in _CACHE:
        _CACHE["nc"] = _build()
    nc = _CACHE["nc"]

    A64 = (Wq.astype(np.float64) @ Wk.astype(np.float64).T) / np.sqrt(H)
    cb64 = (Wk.astype(np.float64) @ bq.astype(np.float64)) / np.sqrt(H)
    a_np = A64.astype(BF16)
    cb_np = np.ascontiguousarray(cb64.astype(np.float32).reshape(2, 128).T)
    r = np.arange(128)[:, None]
    c = np.arange(384)[None, :]
    mk_np = (np.abs(c - r - HALO) <= W).astype(BF16)

    in_maps = []
    for core in range(8):
        b, h = core // 2, core % 2
        q0 = h * SH
        xpad = np.zeros((NK, H), np.float32)
        lo, hi = q0 - HALO, q0 + SH + HALO
        slo, shi = max(lo, 0), min(hi, S)
        xpad[slo - lo: shi - lo, :] = x[b, slo:shi, :]
        xn_np = xpad.astype(BF16)
        xT_np = np.ascontiguousarray(xpad.T).astype(BF16)
        rc_np = np.zeros((128, NQB), np.float32)
        rr = np.arange(128)
        if h == 0:
            rc_np[:, 0] = -(128 - rr).astype(np.float32)   # padded keys j<0
        else:
            rc_np[:, NQB - 1] = -(rr + 1).astype(np.float32)  # padded keys j>=S
        in_maps.append({
            "xT": xT_np, "xn": xn_np, "a": a_np, "cb": cb_np,
            "mk": mk_np, "rc": rc_np,
        })

    import os
    trace = bool(os.environ.get("BASS_TRACE"))
    res = run_bass_kernel_spmd(nc, in_maps, list(range(8)), trace=trace)
    _CACHE["last"] = res

    out = np.zeros((B, H), np.float64)
    for b in range(B):
        u = (res.results[2 * b]["u"][0].astype(np.float64)
             + res.results[2 * b + 1]["u"][0].astype(np.float64))
        out[b] = (u / S) @ Wv.astype(np.float64) + bv
    return out.astype(np.float32)


# revision 14
# speedup vs baseline: 1.0032x; 1.0032x over previous
"""Local (banded) attention -> mean over sequence, on 8 TRN2 NeuronCores.

Math: out[b] = mean_i softmax_j(masked(q_i . k_j / sqrt(H)))-weighted v_j
Reductions used (exact up to softmax shift invariance):
  1. scores'[i,j] = qa_i . x_j with qa = x @ A + cb,
     A = Wq Wk^T / sqrt(H), cb = Wk bq / sqrt(H)
     (terms constant in j drop out of the softmax).
  2. mean_i ctx_i = (1/S) sum_j tw_j v_j with tw_j = sum_i w_ij, and since
     sum_j tw_j = S:  out = (u/S) @ Wv + bv with u = sum_j tw_j x_j.
So the device kernel only computes qa, banded exp-scores, per-key total
weights tw, and u = tw @ x_slice.  The [4,256]@[256,256] epilogue runs on host.

Sharding: 8 cores = batch(4) x sequence-half(2); each core owns 2048 query
rows and a symmetric 128-row halo key range (zero-padded outside the
sequence).  Zero-padded keys contribute exp(0)=1 to each edge query's row
sum; that count is exact and is subtracted via the reduce-init operand of
tensor_tensor_reduce.  Padded keys contribute 0 to u (x row is 0), so the
result is exact.  Per-core partial u vectors sum on the host (u is linear
in tw).
"""

import numpy as np
import ml_dtypes

B, S, H = 4, 4096, 256
W = 128          # window size this kernel is specialized for
SH = S // 2      # query rows per core
HALO = 128
NK = SH + 2 * HALO   # keys per core incl. zero-padded halo
NKC = NK // 128      # 18 key chunks
NQB = SH // 128      # 16 query blocks
BF16 = ml_dtypes.bfloat16

_CACHE = {}


def _build():
    import concourse.bass as bass
    import concourse.tile as tile
    import concourse.mybir as mybir
    from concourse import bacc

    f32 = mybir.dt.float32
    bf16 = mybir.dt.bfloat16

    nc = bacc.Bacc(
        "TRN2", target_bir_lowering=False, debug=False,
        enable_asserts=False, num_devices=1,
    )

    xT_d = nc.dram_tensor("xT", [H, NK], bf16, kind="ExternalInput").ap()
    xn_d = nc.dram_tensor("xn", [NK, H], bf16, kind="ExternalInput").ap()
    a_d = nc.dram_tensor("a", [H, H], bf16, kind="ExternalInput").ap()
    cb_d = nc.dram_tensor("cb", [128, 2], f32, kind="ExternalInput").ap()
    mk_d = nc.dram_tensor("mk", [128, 384], bf16, kind="ExternalInput").ap()
    rc_d = nc.dram_tensor("rc", [128, NQB], f32, kind="ExternalInput").ap()
    u_d = nc.dram_tensor("u", [1, 256], f32, kind="ExternalOutput").ap()

    with tile.TileContext(nc) as tc:
        with (
            tc.tile_pool(name="cst", bufs=1) as cst,
            tc.tile_pool(name="big", bufs=1) as big,
            tc.tile_pool(name="wrk", bufs=4) as wrk,
            tc.tile_pool(name="pqa", bufs=1, space="PSUM") as pqa,
            tc.tile_pool(name="psc", bufs=4, space="PSUM") as psc,
            tc.tile_pool(name="ptw", bufs=1, space="PSUM") as ptw,
            tc.tile_pool(name="pu", bufs=1, space="PSUM") as pu,
        ):
            a0 = cst.tile([128, 256], bf16, tag="a0")
            a1 = cst.tile([128, 256], bf16, tag="a1")
            cb_sb = cst.tile([128, 2], f32, tag="cb")
            mk_sb = cst.tile([128, 384], bf16, tag="mk")
            rc_sb = cst.tile([128, NQB], f32, tag="rc")
            xT0 = big.tile([128, NK], bf16, tag="xT0")
            xT1 = big.tile([128, NK], bf16, tag="xT1")
            xn_sb = big.tile([128, NKC * 256], bf16, tag="xn")
            qa0 = big.tile([128, SH], bf16, tag="qa0")
            qa1 = big.tile([128, SH], bf16, tag="qa1")
            twT_sb = cst.tile([128, NKC], bf16, tag="twT")
            u_sb = cst.tile([1, 256], f32, tag="u")

            nc.sync.dma_start(a0[:], a_d[0:128, :])
            nc.sync.dma_start(a1[:], a_d[128:256, :])
            nc.sync.dma_start(cb_sb[:], cb_d[:])
            nc.sync.dma_start(mk_sb[:], mk_d[:])
            nc.sync.dma_start(rc_sb[:], rc_d[:])
            ones_bf = cst.tile([128, 1], bf16, tag="ones")
            nc.gpsimd.memset(ones_bf[:], 1.0)
            nc.sync.dma_start(xT0[:], xT_d[0:128, :])
            nc.sync.dma_start(xT1[:], xT_d[128:256, :])
            xn_v = xn_sb.rearrange("p (c d) -> p c d", d=256)
            xnd_v = xn_d.rearrange("(c p) d -> p c d", p=128)
            for g in range(3):
                nc.sync.dma_start(xn_v[:, 6 * g:6 * (g + 1), :],
                                  xnd_v[:, 6 * g:6 * (g + 1), :])

            qa = (qa0, qa1)
            xT = (xT0, xT1)
            a = (a0, a1)

            # qa projection: qaT[m] = sum_k A[k, m-chunk].T @ xT[k][:, queries]
            # (m, k) fixed across n pairs so each weight loads twice, and the
            # qa phase only holds 2 PSUM banks (attention pipeline gets 4)
            for m in range(2):
                for np_ in range(2):
                    pss = [pqa.tile([128, 512], f32, tag=f"pqa{n}",
                                    name=f"pqa{m}_{np_}_{n}")
                           for n in range(2)]
                    for k in range(2):
                        for n in range(2):
                            nn = np_ * 2 + n
                            nc.tensor.matmul(
                                pss[n][:],
                                a[k][:, m * 128:(m + 1) * 128],
                                xT[k][:, HALO + nn * 512: HALO + (nn + 1) * 512],
                                start=(k == 0), stop=(k == 1),
                            )
                    for n in range(2):
                        nn = np_ * 2 + n
                        nc.scalar.activation(
                            qa[m][:, nn * 512:(nn + 1) * 512], pss[n][:],
                            mybir.ActivationFunctionType.Identity,
                            bias=cb_sb[:, m:m + 1],
                        )

            twp = ptw.tile([128, NKC], f32, tag="tw")
            up = pu.tile([1, 256], f32, tag="u")
            # query block i attends local key band [128*i, 128*i+384)
            em_live = {}
            ivb_live = {}

            def emit_chunk(jc):
                # key chunk jc accumulates from blocks jc-2..jc; the group
                # opens and closes before the next chunk's group starts
                # (sim requires one pending PSUM group per zero region).
                blocks = [i for i in range(jc - 2, jc + 1) if 0 <= i < NQB]
                for i in blocks:
                    nc.tensor.matmul(
                        twp[:, jc:jc + 1],
                        em_live[i][:, (jc - i) * 128:(jc - i + 1) * 128],
                        ones_bf[:],
                        start=(i == blocks[0]), stop=(i == blocks[-1]),
                    )
                if jc % 3 == 2 or jc == NKC - 1:
                    g0 = (jc // 3) * 3
                    nc.scalar.copy(twT_sb[:, g0:jc + 1], twp[:, g0:jc + 1])
                    for j2 in range(g0, jc + 1):
                        nc.tensor.matmul(
                            up[:],
                            twT_sb[:, j2:j2 + 1],
                            xn_sb[:, j2 * 256:(j2 + 1) * 256],
                            start=(j2 == 0), stop=(j2 == NKC - 1),
                        )

            rs_all = cst.tile([128, NQB], f32, tag="rs_all")
            iv_all = cst.tile([128, NQB], f32, tag="iv_all")
            GB = 4  # reciprocal batch
            for g in range(NQB // GB):
                ems = {}
                for i in range(g * GB, (g + 1) * GB):
                    c0 = 128 * i
                    ps = psc.tile([128, 384], f32, tag="psc")
                    for k in range(2):
                        nc.tensor.matmul(
                            ps[:],
                            qa[k][:, i * 128:(i + 1) * 128],
                            xT[k][:, c0:c0 + 384],
                            start=(k == 0), stop=(k == 1),
                        )
                    ex = wrk.tile([128, 384], bf16, tag="ex")
                    nc.scalar.activation(
                        ex[:], ps[:], mybir.ActivationFunctionType.Exp,
                    )
                    em = wrk.tile([128, 384], bf16, tag=f"em{i % (GB + 1)}",
                                  name=f"em_{i}")
                    rs0 = wrk.tile([128, 1], f32, tag="rs0")
                    nc.vector.scalar_tensor_tensor(
                        em[:], ex[:], 1.0, mk_sb[:],
                        mybir.AluOpType.mult, mybir.AluOpType.mult,
                        accum_out=rs0[:],
                    )
                    nc.vector.tensor_scalar_add(
                        rs_all[:, i:i + 1], rs0[:], rc_sb[:, i:i + 1])
                    ems[i] = em
                gs = slice(g * GB, (g + 1) * GB)
                nc.vector.reciprocal(iv_all[:, gs], rs_all[:, gs])
                for i in range(g * GB, (g + 1) * GB):
                    emn = wrk.tile([128, 384], bf16, tag=f"emn{i % (GB + 1)}",
                                   name=f"emn_{i}")
                    nc.vector.tensor_scalar_mul(
                        emn[:], ems[i][:], iv_all[:, i:i + 1])
                    em_live[i] = emn
                for i in range(g * GB, (g + 1) * GB):
                    emit_chunk(i)
                    if i == NQB - 1:
                        emit_chunk(i + 1)
                        emit_chunk(i + 2)

            nc.scalar.copy(u_sb[:], up[:])
            nc.sync.dma_start(u_d[:], u_sb[:])

    nc.compile()
    return nc


def _numpy_fallback(x, Wq, bq, Wk, bk, Wv, bv, window_size):
    out = np.zeros((B, H), np.float64)
    xs = x.astype(np.float64)
    A = (Wq.astype(np.float64) @ Wk.astype(np.float64).T) / np.sqrt(H)
    cb = (Wk.astype(np.float64) @ bq.astype(np.float64)) / np.sqrt(H)
    idx = np.arange(x.shape[1])
    band = np.abs(idx[:, None] - idx[None, :]) <= int(window_size)
    for b in range(x.shape[0]):
        qa = xs[b] @ A + cb
        sc = qa @ xs[b].T
        e = np.exp(sc - sc.max(axis=-1, keepdims=True)) * band
        w = e / e.sum(-1, keepdims=True)
        tw = w.sum(axis=0)
        out[b] = (tw @ xs[b] / x.shape[1]) @ Wv.astype(np.float64) + bv
    return out.astype(np.float32)


def kernel(x, Wq, bq, Wk, bk, Wv, bv, window_size):
    x = np.asarray(x)
    Wq, bq = np.asarray(Wq), np.asarray(bq)
    Wk, bk = np.asarray(Wk), np.asarray(bk)
    Wv, bv = np.asarray(Wv), np.asarray(bv)
    if int(window_size) != W or x.shape != (B, S, H):
        return _numpy_fallback(x, Wq, bq, Wk, bk, Wv, bv, window_size)

    from concourse.bass_utils import run_bass_kernel_spmd

    if "nc" not in _CACHE:
        _CACHE["nc"] = _build()
    nc = _CACHE["nc"]

    A64 = (Wq.astype(np.float64) @ Wk.astype(np.float64).T) / np.sqrt(H)
    cb64 = (Wk.astype(np.float64) @ bq.astype(np.float64)) / np.sqrt(H)
    a_np = A64.astype(BF16)
    cb_np = np.ascontiguousarray(cb64.astype(np.float32).reshape(2, 128).T)
    r = np.arange(128)[:, None]
    c = np.arange(384)[None, :]
    mk_np = (np.abs(c - r - HALO) <= W).astype(BF16)

    in_maps = []
    for core in range(8):
        b, h = core // 2, core % 2
        q0 = h * SH
        xpad = np.zeros((NK, H), np.float32)
        lo, hi = q0 - HALO, q0 + SH + HALO
        slo, shi = max(lo, 0), min(hi, S)
        xpad[slo - lo: shi - lo, :] = x[b, slo:shi, :]
        xn_np = xpad.astype(BF16)
        xT_np = np.ascontiguousarray(xpad.T).astype(BF16)
        rc_np = np.zeros((128, NQB), np.float32)
        rr = np.arange(128)
        if h == 0:
            rc_np[:, 0] = -(128 - rr).astype(np.float32)   # padded keys j<0
        else:
            rc_np[:, NQB - 1] = -(rr + 1).astype(np.float32)  # padded keys j>=S
        in_maps.append({
            "xT": xT_np, "xn": xn_np, "a": a_np, "cb": cb_np,
            "mk": mk_np, "rc": rc_np,
        })

    import os
    trace = bool(os.environ.get("BASS_TRACE"))
    res = run_bass_kernel_spmd(nc, in_maps, list(range(8)), trace=trace)
    _CACHE["last"] = res

    out = np.zeros((B, H), np.float64)
    for b in range(B):
        u = (res.results[2 * b]["u"][0].astype(np.float64)
             + res.results[2 * b + 1]["u"][0].astype(np.float64))
        out[b] = (u / S) @ Wv.astype(np.float64) + bv
    return out.astype(np.float32)


# revision 15
# speedup vs baseline: 1.0272x; 1.0239x over previous
"""Local (banded) attention -> mean over sequence, on 8 TRN2 NeuronCores.

Math: out[b] = mean_i softmax_j(masked(q_i . k_j / sqrt(H)))-weighted v_j
Reductions used (exact up to softmax shift invariance):
  1. scores'[i,j] = qa_i . x_j with qa = x @ A + cb,
     A = Wq Wk^T / sqrt(H), cb = Wk bq / sqrt(H)
     (terms constant in j drop out of the softmax).
  2. mean_i ctx_i = (1/S) sum_j tw_j v_j with tw_j = sum_i w_ij, and since
     sum_j tw_j = S:  out = (u/S) @ Wv + bv with u = sum_j tw_j x_j.
So the device kernel only computes qa, banded exp-scores, per-key total
weights tw, and u = tw @ x_slice.  The [4,256]@[256,256] epilogue runs on host.

Sharding: 8 cores = batch(4) x sequence-half(2); each core owns 2048 query
rows and a symmetric 128-row halo key range (zero-padded outside the
sequence).  Zero-padded keys contribute exp(0)=1 to each edge query's row
sum; that count is exact and is subtracted via the reduce-init operand of
tensor_tensor_reduce.  Padded keys contribute 0 to u (x row is 0), so the
result is exact.  Per-core partial u vectors sum on the host (u is linear
in tw).
"""

import numpy as np
import ml_dtypes

B, S, H = 4, 4096, 256
W = 128          # window size this kernel is specialized for
SH = S // 2      # query rows per core
HALO = 128
NK = SH + 2 * HALO   # keys per core incl. zero-padded halo
NKC = NK // 128      # 18 key chunks
NQB = SH // 128      # 16 query blocks
BF16 = ml_dtypes.bfloat16

_CACHE = {}


def _build():
    import concourse.bass as bass
    import concourse.tile as tile
    import concourse.mybir as mybir
    from concourse import bacc

    f32 = mybir.dt.float32
    bf16 = mybir.dt.bfloat16

    nc = bacc.Bacc(
        "TRN2", target_bir_lowering=False, debug=False,
        enable_asserts=False, num_devices=1,
    )

    xT_d = nc.dram_tensor("xT", [H, NK], bf16, kind="ExternalInput").ap()
    xn_d = nc.dram_tensor("xn", [NK, H], bf16, kind="ExternalInput").ap()
    a_d = nc.dram_tensor("a", [H, H], bf16, kind="ExternalInput").ap()
    cb_d = nc.dram_tensor("cb", [128, 2], f32, kind="ExternalInput").ap()
    mk_d = nc.dram_tensor("mk", [128, 384], bf16, kind="ExternalInput").ap()
    rc_d = nc.dram_tensor("rc", [128, NQB], f32, kind="ExternalInput").ap()
    u_d = nc.dram_tensor("u", [1, 256], f32, kind="ExternalOutput").ap()

    with tile.TileContext(nc) as tc:
        with (
            tc.tile_pool(name="cst", bufs=1) as cst,
            tc.tile_pool(name="big", bufs=1) as big,
            tc.tile_pool(name="wrk", bufs=4) as wrk,
            tc.tile_pool(name="pqa", bufs=1, space="PSUM") as pqa,
            tc.tile_pool(name="psc", bufs=4, space="PSUM") as psc,
            tc.tile_pool(name="ptw", bufs=1, space="PSUM") as ptw,
            tc.tile_pool(name="pu", bufs=1, space="PSUM") as pu,
        ):
            a0 = cst.tile([128, 256], bf16, tag="a0")
            a1 = cst.tile([128, 256], bf16, tag="a1")
            cb_sb = cst.tile([128, 2], f32, tag="cb")
            mk_sb = cst.tile([128, 384], bf16, tag="mk")
            rc_sb = cst.tile([128, NQB], f32, tag="rc")
            xT0 = big.tile([128, NK], bf16, tag="xT0")
            xT1 = big.tile([128, NK], bf16, tag="xT1")
            xn_sb = big.tile([128, NKC * 256], bf16, tag="xn")
            qa0 = big.tile([128, SH], bf16, tag="qa0")
            qa1 = big.tile([128, SH], bf16, tag="qa1")
            twT_sb = cst.tile([128, NKC], bf16, tag="twT")
            u_sb = cst.tile([1, 256], f32, tag="u")

            nc.sync.dma_start(a0[:], a_d[0:128, :])
            nc.sync.dma_start(a1[:], a_d[128:256, :])
            nc.sync.dma_start(cb_sb[:], cb_d[:])
            nc.sync.dma_start(mk_sb[:], mk_d[:])
            nc.sync.dma_start(rc_sb[:], rc_d[:])
            ones_bf = cst.tile([128, 1], bf16, tag="ones")
            nc.gpsimd.memset(ones_bf[:], 1.0)
            nc.sync.dma_start(xT0[:], xT_d[0:128, :])
            nc.sync.dma_start(xT1[:], xT_d[128:256, :])
            xn_v = xn_sb.rearrange("p (c d) -> p c d", d=256)
            xnd_v = xn_d.rearrange("(c p) d -> p c d", p=128)
            for g in range(3):
                nc.sync.dma_start(xn_v[:, 6 * g:6 * (g + 1), :],
                                  xnd_v[:, 6 * g:6 * (g + 1), :])

            qa = (qa0, qa1)
            xT = (xT0, xT1)
            a = (a0, a1)

            # qa projection: qaT[m] = sum_k A[k, m-chunk].T @ xT[k][:, queries]
            # (m, k) fixed across n pairs so each weight loads twice, and the
            # qa phase only holds 2 PSUM banks (attention pipeline gets 4)
            for m in range(2):
                for np_ in range(2):
                    pss = [pqa.tile([128, 512], f32, tag=f"pqa{n}",
                                    name=f"pqa{m}_{np_}_{n}")
                           for n in range(2)]
                    for k in range(2):
                        for n in range(2):
                            nn = np_ * 2 + n
                            nc.tensor.matmul(
                                pss[n][:],
                                a[k][:, m * 128:(m + 1) * 128],
                                xT[k][:, HALO + nn * 512: HALO + (nn + 1) * 512],
                                start=(k == 0), stop=(k == 1),
                            )
                    for n in range(2):
                        nn = np_ * 2 + n
                        nc.vector.tensor_scalar_add(
                            qa[m][:, nn * 512:(nn + 1) * 512], pss[n][:],
                            cb_sb[:, m:m + 1],
                        )

            twp = ptw.tile([128, NKC], f32, tag="tw")
            up = pu.tile([1, 256], f32, tag="u")
            # query block i attends local key band [128*i, 128*i+384)
            em_live = {}
            ivb_live = {}

            def emit_chunk(jc):
                # key chunk jc accumulates from blocks jc-2..jc; the group
                # opens and closes before the next chunk's group starts
                # (sim requires one pending PSUM group per zero region).
                blocks = [i for i in range(jc - 2, jc + 1) if 0 <= i < NQB]
                for i in blocks:
                    nc.tensor.matmul(
                        twp[:, jc:jc + 1],
                        em_live[i][:, (jc - i) * 128:(jc - i + 1) * 128],
                        ones_bf[:],
                        start=(i == blocks[0]), stop=(i == blocks[-1]),
                    )
                if jc % 3 == 2 or jc == NKC - 1:
                    g0 = (jc // 3) * 3
                    nc.scalar.copy(twT_sb[:, g0:jc + 1], twp[:, g0:jc + 1])
                    for j2 in range(g0, jc + 1):
                        nc.tensor.matmul(
                            up[:],
                            twT_sb[:, j2:j2 + 1],
                            xn_sb[:, j2 * 256:(j2 + 1) * 256],
                            start=(j2 == 0), stop=(j2 == NKC - 1),
                        )

            rs_all = cst.tile([128, NQB], f32, tag="rs_all")
            iv_all = cst.tile([128, NQB], f32, tag="iv_all")
            GB = 4  # reciprocal batch
            for g in range(NQB // GB):
                ems = {}
                for i in range(g * GB, (g + 1) * GB):
                    c0 = 128 * i
                    ps = psc.tile([128, 384], f32, tag="psc")
                    for k in range(2):
                        nc.tensor.matmul(
                            ps[:],
                            qa[k][:, i * 128:(i + 1) * 128],
                            xT[k][:, c0:c0 + 384],
                            start=(k == 0), stop=(k == 1),
                        )
                    ex = wrk.tile([128, 384], bf16, tag="ex")
                    nc.scalar.activation(
                        ex[:], ps[:], mybir.ActivationFunctionType.Exp,
                    )
                    em = wrk.tile([128, 384], bf16, tag=f"em{i % (GB + 1)}",
                                  name=f"em_{i}")
                    rs0 = wrk.tile([128, 1], f32, tag="rs0")
                    nc.vector.scalar_tensor_tensor(
                        em[:], ex[:], 1.0, mk_sb[:],
                        mybir.AluOpType.mult, mybir.AluOpType.mult,
                        accum_out=rs0[:],
                    )
                    nc.vector.tensor_scalar_add(
                        rs_all[:, i:i + 1], rs0[:], rc_sb[:, i:i + 1])
                    ems[i] = em
                gs = slice(g * GB, (g + 1) * GB)
                nc.vector.reciprocal(iv_all[:, gs], rs_all[:, gs])
                for i in range(g * GB, (g + 1) * GB):
                    emn = wrk.tile([128, 384], bf16, tag=f"emn{i % (GB + 1)}",
                                   name=f"emn_{i}")
                    nc.vector.tensor_scalar_mul(
                        emn[:], ems[i][:], iv_all[:, i:i + 1])
                    em_live[i] = emn
                for i in range(g * GB, (g + 1) * GB):
                    emit_chunk(i)
                    if i == NQB - 1:
                        emit_chunk(i + 1)
                        emit_chunk(i + 2)

            nc.scalar.copy(u_sb[:], up[:])
            nc.sync.dma_start(u_d[:], u_sb[:])

    nc.compile()
    return nc


def _numpy_fallback(x, Wq, bq, Wk, bk, Wv, bv, window_size):
    out = np.zeros((B, H), np.float64)
    xs = x.astype(np.float64)
    A = (Wq.astype(np.float64) @ Wk.astype(np.float64).T) / np.sqrt(H)
    cb = (Wk.astype(np.float64) @ bq.astype(np.float64)) / np.sqrt(H)
    idx = np.arange(x.shape[1])
    band = np.abs(idx[:, None] - idx[None, :]) <= int(window_size)
    for b in range(x.shape[0]):
        qa = xs[b] @ A + cb
        sc = qa @ xs[b].T
        e = np.exp(sc - sc.max(axis=-1, keepdims=True)) * band
        w = e / e.sum(-1, keepdims=True)
        tw = w.sum(axis=0)
        out[b] = (tw @ xs[b] / x.shape[1]) @ Wv.astype(np.float64) + bv
    return out.astype(np.float32)


def kernel(x, Wq, bq, Wk, bk, Wv, bv, window_size):
    x = np.asarray(x)
    Wq, bq = np.asarray(Wq), np.asarray(bq)
    Wk, bk = np.asarray(Wk), np.asarray(bk)
    Wv, bv = np.asarray(Wv), np.asarray(bv)
    if int(window_size) != W or x.shape != (B, S, H):
        return _numpy_fallback(x, Wq, bq, Wk, bk, Wv, bv, window_size)

    from concourse.bass_utils import run_bass_kernel_spmd

    if "nc" not in _CACHE:
        _CACHE["nc"] = _build()
    nc = _CACHE["nc"]

    A64 = (Wq.astype(np.float64) @ Wk.astype(np.float64).T) / np.sqrt(H)
    cb64 = (Wk.astype(np.float64) @ bq.astype(np.float64)) / np.sqrt(H)
    a_np = A64.astype(BF16)
    cb_np = np.ascontiguousarray(cb64.astype(np.float32).reshape(2, 128).T)
    r = np.arange(128)[:, None]
    c = np.arange(384)[None, :]
    mk_np = (np.abs(c - r - HALO) <= W).astype(BF16)

    in_maps = []
    for core in range(8):
        b, h = core // 2, core % 2
        q0 = h * SH
        xpad = np.zeros((NK, H), np.float32)
        lo, hi = q0 - HALO, q0 + SH + HALO
        slo, shi = max(lo, 0), min(hi, S)
        xpad[slo - lo: shi - lo, :] = x[b, slo:shi, :]
        xn_np = xpad.astype(BF16)
        xT_np = np.ascontiguousarray(xpad.T).astype(BF16)
        rc_np = np.zeros((128, NQB), np.float32)
        rr = np.arange(128)
        if h == 0:
            rc_np[:, 0] = -(128 - rr).astype(np.float32)   # padded keys j<0
        else:
            rc_np[:, NQB - 1] = -(rr + 1).astype(np.float32)  # padded keys j>=S
        in_maps.append({
            "xT": xT_np, "xn": xn_np, "a": a_np, "cb": cb_np,
            "mk": mk_np, "rc": rc_np,
        })

    import os
    trace = bool(os.environ.get("BASS_TRACE"))
    res = run_bass_kernel_spmd(nc, in_maps, list(range(8)), trace=trace)
    _CACHE["last"] = res

    out = np.zeros((B, H), np.float64)
    for b in range(B):
        u = (res.results[2 * b]["u"][0].astype(np.float64)
             + res.results[2 * b + 1]["u"][0].astype(np.float64))
        out[b] = (u / S) @ Wv.astype(np.float64) + bv
    return out.astype(np.float32)
